# revision 2
# baseline (speedup 1.0000x reference)
"""Peephole-LSTM Trainium2 kernel: 8-way data-parallel over batch.

Per core (batch shard of 32):
  - h, c kept on-chip in [H-on-partitions] layout: tile [128, 2*32],
    column (j*32+b) holds element h[128*j+p] for batch b.
  - Gate preacts via PE: for each of 8 M-blocks (4 gates x 2 H-halves),
    accumulate 3 K-chunks (W rows 0:128, W rows 128:256, U rows 0:128)
    with stationary bf16 weight blocks and moving h/x tiles [128, 32].
  - Peepholes fused into DVE scalar_tensor_tensor: (c * V) + psum.
  - Sigmoid/Tanh on ACT; cell update and h on DVE.
  - x is pre-transposed/cast to bf16 on host; hidden_seq is written as
    hsT[t, p, j*32+b] and untransposed on host.
"""
import sys, os
sys.path.insert(0, '/opt/trn_rl_repo')
import numpy as np
import ml_dtypes
from contextlib import ExitStack

import concourse.bass as bass
import concourse.mybir as mybir
import concourse.tile as tile
from concourse.bass_utils import run_bass_kernel_spmd

B, T, I, H = 256, 512, 128, 256
NC = 8
BL = B // NC      # 32 batch rows per core
NH = 2            # H halves of 128
f32 = mybir.dt.float32
bf16 = mybir.dt.bfloat16
AF = mybir.ActivationFunctionType
ALU = mybir.AluOpType

T_STEPS = int(os.environ.get("LSTM_T_OVERRIDE", T))
X_CHUNK = 32      # timesteps of x per input DMA


def _split_excess_waits(nc):
    """This container's walrus lowers Drain to TPB_CTRL with room for a
    single sync-wait; fan extra waits out onto preceding drains."""
    n = 0
    for f in nc.m.functions:
        for blk in f.blocks:
            insts = blk.instructions
            i = 0
            while i < len(insts):
                ins = insts[i]
                si = getattr(ins, 'sync_info', None)
                cap = 1
                if si is not None and si.on_wait and len(si.on_wait) > cap:
                    waits = list(si.on_wait)
                    si.on_wait = waits[:cap]
                    rest = waits[cap:]
                    k = 0
                    while rest:
                        chunk, rest = rest[:1], rest[1:]
                        pre = mybir.InstDrain(
                            name=f"{ins.name}-wsplit{k}", ins=[], outs=[],
                            engine=ins.engine,
                            sync_info=mybir.SyncInfo(on_wait=chunk, on_update=[]))
                        insts.insert(i, pre)
                        i += 1
                        k += 1
                        n += 1
                i += 1
    return n


def _build(with_bias):
    nc = bass.Bass()
    xt = nc.dram_tensor("xt", [T_STEPS, I, BL], bf16, kind="ExternalInput")
    wu = nc.dram_tensor("wu", [8, 3, 128, 128], bf16, kind="ExternalInput")
    vv = nc.dram_tensor("vv", [128, 6], f32, kind="ExternalInput")
    bb = nc.dram_tensor("bb", [128, 8], f32, kind="ExternalInput")
    hsT = nc.dram_tensor("hsT", [T_STEPS, 128, NH * BL], f32, kind="ExternalOutput")
    cT = nc.dram_tensor("cT", [128, NH * BL], f32, kind="ExternalOutput")

    W = NH * BL  # 64: width of one [all-H, batch] tile

    with ExitStack() as ctx:
        tc = ctx.enter_context(tile.TileContext(nc))
        const = ctx.enter_context(tc.tile_pool(name="const", bufs=1))
        xpool = ctx.enter_context(tc.tile_pool(name="xp", bufs=3))
        hpool = ctx.enter_context(tc.tile_pool(name="hp", bufs=3))
        cpool = ctx.enter_context(tc.tile_pool(name="cp", bufs=3))
        work = ctx.enter_context(tc.tile_pool(name="wk", bufs=3))
        opool = ctx.enter_context(tc.tile_pool(name="op", bufs=6))
        psum = ctx.enter_context(tc.tile_pool(name="ps", bufs=4, space="PSUM"))

        wu_sb = const.tile([128, 24, 128], bf16)
        nc.sync.dma_start(out=wu_sb, in_=wu.rearrange("m k p c -> p (m k) c"))
        vv_sb = const.tile([128, 6], f32)
        nc.sync.dma_start(out=vv_sb, in_=vv[:, :])
        bb_sb = const.tile([128, 8], f32)
        nc.sync.dma_start(out=bb_sb, in_=bb[:, :])

        h_bf = hpool.tile([128, W], bf16)
        nc.vector.memset(h_bf, 0)
        c = cpool.tile([128, W], f32)
        nc.vector.memset(c, 0)

        xc = None
        for t in range(T_STEPS):
            if t % X_CHUNK == 0:
                n = min(X_CHUNK, T_STEPS - t)
                xc = xpool.tile([128, X_CHUNK, BL], bf16)
                nc.sync.dma_start(
                    out=xc[:, :n, :],
                    in_=xt[t:t + n].rearrange("t p b -> p t b"))

            P = psum.tile([128, 8 * BL], f32)
            # M-block order: f0 f1 i0 i1 (head), then c0 c1 o0 o1
            for m in range(8):
                for kc in range(3):
                    rhs = (h_bf[:, kc * BL:(kc + 1) * BL] if kc < 2
                           else xc[:, t % X_CHUNK, :])
                    nc.tensor.matmul(
                        P[:, m * BL:(m + 1) * BL],
                        wu_sb[:, m * 3 + kc, :], rhs,
                        start=(kc == 0), stop=(kc == 2))

            # pre_f/pre_i = (c * V) + psum, fused on DVE
            prefi = work.tile([128, 4 * BL], f32)
            for g in range(2):          # 0=f, 1=i
                for j in range(NH):
                    m = g * 2 + j
                    nc.vector.scalar_tensor_tensor(
                        out=prefi[:, m * BL:(m + 1) * BL],
                        in0=c[:, j * BL:(j + 1) * BL],
                        scalar=vv_sb[:, m:m + 1],
                        in1=P[:, m * BL:(m + 1) * BL],
                        op0=ALU.mult, op1=ALU.add)
            fi = work.tile([128, 4 * BL], f32)
            if with_bias:
                for g in range(2):
                    for j in range(NH):
                        m = g * 2 + j
                        nc.scalar.activation(
                            fi[:, m * BL:(m + 1) * BL],
                            prefi[:, m * BL:(m + 1) * BL],
                            AF.Sigmoid, bias=bb_sb[:, m:m + 1])
            else:
                nc.scalar.activation(fi, prefi, AF.Sigmoid)

            cbar = work.tile([128, W], f32)
            if with_bias:
                for j in range(NH):
                    nc.scalar.activation(
                        cbar[:, j * BL:(j + 1) * BL],
                        P[:, (4 + j) * BL:(5 + j) * BL],
                        AF.Tanh, bias=bb_sb[:, 4 + j:5 + j])
            else:
                nc.scalar.activation(cbar, P[:, 4 * BL:6 * BL], AF.Tanh)

            t1 = work.tile([128, W], f32)
            nc.vector.tensor_mul(t1, fi[:, 0:W], c)
            t2 = work.tile([128, W], f32)
            nc.vector.tensor_mul(t2, fi[:, W:2 * W], cbar)
            c = cpool.tile([128, W], f32)
            nc.vector.tensor_add(c, t1, t2)

            preo = work.tile([128, W], f32)
            for j in range(NH):
                nc.vector.scalar_tensor_tensor(
                    out=preo[:, j * BL:(j + 1) * BL],
                    in0=c[:, j * BL:(j + 1) * BL],
                    scalar=vv_sb[:, 4 + j:5 + j],
                    in1=P[:, (6 + j) * BL:(7 + j) * BL],
                    op0=ALU.mult, op1=ALU.add)
            th = work.tile([128, W], f32)
            nc.scalar.activation(th, c, AF.Tanh)
            o = work.tile([128, W], f32)
            if with_bias:
                for j in range(NH):
                    nc.scalar.activation(
                        o[:, j * BL:(j + 1) * BL],
                        preo[:, j * BL:(j + 1) * BL],
                        AF.Sigmoid, bias=bb_sb[:, 6 + j:7 + j])
            else:
                nc.scalar.activation(o, preo, AF.Sigmoid)

            hf = opool.tile([128, W], f32)
            nc.vector.tensor_mul(hf, o, th)
            h_bf = hpool.tile([128, W], bf16)
            nc.vector.tensor_copy(h_bf, hf)
            nc.sync.dma_start(out=hsT[t], in_=hf)

        cout = opool.tile([128, W], f32)
        nc.vector.tensor_copy(cout, c)
        nc.sync.dma_start(out=cT[:, :], in_=cout)

    _split_excess_waits(nc)
    return nc


def _pack_host(x, U_f, W_f, U_i, W_i, U_c, W_c, U_o, W_o,
               V_f, V_i, V_o, b_f, b_i, b_c, b_o):
    x = np.asarray(x, dtype=np.float32)
    Ws = [np.asarray(a, np.float32) for a in (W_f, W_i, W_c, W_o)]
    Us = [np.asarray(a, np.float32) for a in (U_f, U_i, U_c, U_o)]
    Vs = [np.asarray(a, np.float32) for a in (V_f, V_i, V_o)]
    bs = [np.asarray(a, np.float32) for a in (b_f, b_i, b_c, b_o)]

    wu = np.zeros((8, 3, 128, 128), np.float32)
    for g in range(4):
        for j in range(NH):
            m = g * 2 + j
            wu[m, 0] = Ws[g][0:128, 128 * j:128 * (j + 1)]
            wu[m, 1] = Ws[g][128:256, 128 * j:128 * (j + 1)]
            wu[m, 2] = Us[g][:, 128 * j:128 * (j + 1)]
    wu = wu.astype(ml_dtypes.bfloat16)

    # vv columns: Vf0 Vf1 Vi0 Vi1 Vo0 Vo1 ; bb columns: f0 f1 i0 i1 c0 c1 o0 o1
    vv = np.stack([Vs[0][0:128], Vs[0][128:256],
                   Vs[1][0:128], Vs[1][128:256],
                   Vs[2][0:128], Vs[2][128:256]], axis=1).astype(np.float32)
    bb = np.stack(sum([[b[0:128], b[128:256]] for b in bs], []),
                  axis=1).astype(np.float32)
    with_bias = bool(np.any(bb != 0.0))

    in_maps = []
    for k in range(NC):
        xk = x[k * BL:(k + 1) * BL, :T_STEPS]          # [BL, T, I]
        xk = np.ascontiguousarray(xk.transpose(1, 2, 0))  # [T, I, BL]
        in_maps.append({
            "xt": xk.astype(ml_dtypes.bfloat16),
            "wu": wu, "vv": vv, "bb": bb,
        })
    return in_maps, with_bias


def kernel(**inputs):
    in_maps, with_bias = _pack_host(**inputs)
    nc = _build(with_bias)
    res = run_bass_kernel_spmd(nc, in_maps, list(range(NC)))

    hidden = np.empty((B, T_STEPS, H), np.float32)
    c_t = np.empty((B, H), np.float32)
    for k in range(NC):
        r = res.results[k]
        hs = r["hsT"].reshape(T_STEPS, 128, NH, BL)      # [t, p, j, b]
        hidden[k * BL:(k + 1) * BL] = hs.transpose(3, 0, 2, 1).reshape(BL, T_STEPS, H)
        cc = r["cT"].reshape(128, NH, BL)
        c_t[k * BL:(k + 1) * BL] = cc.transpose(2, 1, 0).reshape(BL, H)
    h_t = np.ascontiguousarray(hidden[:, -1, :])
    return hidden, h_t, c_t


if __name__ == "__main__":
    rng = np.random.default_rng(0)
    inputs = {"x": rng.standard_normal((B, T, I)).astype(np.float32) * 0.5}
    for name, shp in [("U_f", (I, H)), ("W_f", (H, H)), ("U_i", (I, H)),
                      ("W_i", (H, H)), ("U_c", (I, H)), ("W_c", (H, H)),
                      ("U_o", (I, H)), ("W_o", (H, H))]:
        fan = shp[0] + shp[1]
        inputs[name] = (rng.standard_normal(shp) * np.sqrt(2.0 / fan)).astype(np.float32)
    for name in ["V_f", "V_i", "V_o"]:
        inputs[name] = (rng.standard_normal(H) * 0.1).astype(np.float32)
    for name in ["b_f", "b_i", "b_c", "b_o"]:
        inputs[name] = np.zeros(H, np.float32)

    hidden, h_t, c_t = kernel(**inputs)

    # numpy reference
    h = np.zeros((B, H), np.float32)
    c = np.zeros((B, H), np.float32)
    Tn = T_STEPS
    xs = inputs["x"][:, :Tn]
    sig = lambda z: 1.0 / (1.0 + np.exp(-z))
    hs_ref = np.empty((B, Tn, H), np.float32)
    for t in range(Tn):
        xt_ = xs[:, t]
        f = sig(xt_ @ inputs["U_f"] + h @ inputs["W_f"] + inputs["V_f"] * c + inputs["b_f"])
        i = sig(xt_ @ inputs["U_i"] + h @ inputs["W_i"] + inputs["V_i"] * c + inputs["b_i"])
        c = f * c + i * np.tanh(xt_ @ inputs["U_c"] + h @ inputs["W_c"] + inputs["b_c"])
        o = sig(xt_ @ inputs["U_o"] + h @ inputs["W_o"] + inputs["V_o"] * c + inputs["b_o"])
        h = o * np.tanh(c)
        hs_ref[:, t] = h
    scale = np.abs(hs_ref).max()
    err = np.abs(hidden - hs_ref).max() / scale
    print(f"scale-relative max err hidden: {err:.3e} (scale {scale:.3f})")
    print("c_t err:", np.abs(c_t - c).max() / (np.abs(c).max() + 1e-9))
